# revision 29
# baseline (speedup 1.0000x reference)
"""Multi-head attention (nn_Attention1D) on 8 Trainium2 NeuronCores.

Full inputs in, full output out.  Sharding: batch (2) x head-groups (4 heads
per core).  Each core computes, for its batch b and its 4 heads:
  q = (x_q @ WqT + bq)/sqrt(dk)   (scale folded into weights host-side)
  k = x_k @ WkT + bk
  v = x_v @ WvT + bv
  scoresT[sk,sq] = kT.T-style matmul (keeps softmax axis on partitions)
  p_un = exp(scoresT) * maskT          (exp(s + log(m)) == exp(s)*m)
  xattT_un[dk,sq] = v_ext.T @ p_un     (v_ext has a ones column -> row 64
                                        of the PSUM tile is the softmax
                                        denominator, for free)
  xattT = xattT_un * (1/denom)         (ACT exp(-ln) + K=1 broadcast matmul)
  outT_partial = WoT.T @ xattT         (partial over this core's 256 e-cols)
Host sums the 4 partial outT per batch and adds bo.
"""

import contextlib

import numpy as np

import concourse.bass as bass
import concourse.mybir as mybir
import concourse.tile as tile

F32 = mybir.dt.float32
BF16 = mybir.dt.bfloat16

# ---------------------------------------------------------------- config
import os

F16_P = os.environ.get("ATTN_F16_P", "1") == "1"  # exp/mask/p/v in bf16
F16_X = os.environ.get("ATTN_F16_X", "1") == "1"  # x inputs + qkv weights in bf16

P = 128
NB = 512  # psum bank in fp32 elements == max matmul N


def _split_multiwait(nc, max_waits=1):
    """This walrus build only accepts one sync wait per instruction; hoist
    extra waits onto NoOps inserted just before."""
    for bb in nc.main_func.blocks:
        new_insts = []
        for ins in bb.instructions:
            if ins.sync_info and len(ins.sync_info.on_wait) > max_waits:
                waits = list(ins.sync_info.on_wait)
                ins.sync_info.on_wait = waits[:max_waits]
                for i, w in enumerate(waits[max_waits:]):
                    nop = mybir.InstNoOp(name=f"{ins.name}_ws{i}", ins=[], outs=[])
                    nop.engine = ins.engine
                    nop.sync_info = mybir.SyncInfo(on_wait=[w], on_update=[])
                    nc.register_instruction(nop)
                    new_insts.append(nop)
            new_insts.append(ins)
        bb.instructions = new_insts


def build_program(D=1024, S=2048, E=256, DK=64, CS=512, CQ=1024, f16_p=F16_P, f16_x=F16_X):
    H = E // DK  # heads on this core
    DK1 = DK + 1
    KD = D // P  # k-tiles over model dim
    KE = E // P  # k-tiles over per-core e-slice
    SK = S // P  # partition tiles over key positions
    CQ = min(CQ, S)
    NCS = S // CS
    NCQ = S // CQ
    F32R = mybir.dt.float32r
    XDT = BF16 if f16_x else F32R
    PDT = BF16 if f16_p else F32R
    EXP = mybir.ActivationFunctionType.Exp

    nc = bass.Bass()
    xqT = nc.dram_tensor("xqT", [D, S], XDT, kind="ExternalInput")
    xkT = nc.dram_tensor("xkT", [D, S], XDT, kind="ExternalInput")
    xvT = nc.dram_tensor("xvT", [D, S], XDT, kind="ExternalInput")
    maskT = nc.dram_tensor("maskT", [S, S], PDT, kind="ExternalInput")
    wqT = nc.dram_tensor("wqT", [D, E], XDT, kind="ExternalInput")
    wkT = nc.dram_tensor("wkT", [D, E], XDT, kind="ExternalInput")
    wvT = nc.dram_tensor("wvT", [D, E], XDT, kind="ExternalInput")
    woT = nc.dram_tensor("woT", [E, D], F32R, kind="ExternalInput")
    bqT = nc.dram_tensor("bqT", [E, 1], F32, kind="ExternalInput")
    bkT = nc.dram_tensor("bkT", [E, 1], F32, kind="ExternalInput")
    bv = nc.dram_tensor("bv", [1, E], XDT, kind="ExternalInput")
    ones_r = nc.dram_tensor("ones_r", [P, 64], F32R, kind="ExternalInput")
    ones_x = nc.dram_tensor("ones_x", [1, CS], XDT, kind="ExternalInput")
    ones_p = nc.dram_tensor("ones_p", [1, P], PDT, kind="ExternalInput")
    outT = nc.dram_tensor("outT", [D, S], F32, kind="ExternalOutput")

    xqT_r = xqT.rearrange("(k p) s -> p k s", p=P)
    xkT_r = xkT.rearrange("(k p) s -> p k s", p=P)
    xvT_r = xvT.rearrange("(k p) s -> p k s", p=P)
    maskT_r = maskT.rearrange("(k p) s -> p k s", p=P)
    wqT_r = wqT.rearrange("(k p) e -> p k e", p=P)
    wkT_r = wkT.rearrange("(k p) e -> p k e", p=P)
    wvT_r = wvT.rearrange("(k p) e -> p k e", p=P)
    woT_r = woT.rearrange("(k p) d -> p k d", p=P)
    outT_r = outT.rearrange("(m p) s -> p m s", p=P)

    with tile.TileContext(nc) as tc:
        with (
            tc.tile_pool(name="persist", bufs=1) as persist,
            tc.tile_pool(name="consts", bufs=1) as consts,
        ):
            qT_sb = persist.tile([P, KE, S], F32R)
            kT_sb = persist.tile([P, KE, S], F32R)
            v_sb = persist.tile([P, SK, H, DK1], PDT)
            xattT_sb = persist.tile([P, KE, S], F32R)
            ones_sb = consts.tile([P, 64], F32R)
            onesx_sb = consts.tile([1, CS], XDT)
            bqT_sb = consts.tile([P, KE], F32)
            bkT_sb = consts.tile([P, KE], F32)
            bv_sb = consts.tile([1, E], XDT)
            # memset doesn't support float32r; ones come from DRAM instead.
            # (const DMAs are emitted inside stage A, after the first x chunk,
            # so the SP queue delivers the projection operands first)
            if PDT == BF16:
                nc.gpsimd.memset(v_sb[:, :, :, DK:DK1], 1.0)
            else:
                nc.sync.dma_start(
                    out=v_sb[:, :, :, DK:DK1],
                    in_=bass.AP(
                        tensor=ones_p, offset=0, ap=[[0, P], [0, SK * H], [1, 1]]
                    ),
                )

            # The mask/wo pools are created BEFORE stage A's pools when the
            # bf16-input config leaves enough SBUF: their address zones then
            # don't overlap the x-chunk tiles, so the first mask load (8 MB)
            # runs concurrently with the projections instead of after them.
            _pools = contextlib.ExitStack()

            def _open_bmdw():
                bm = _pools.enter_context(
                    tc.tile_pool(name="bm", bufs=2 if f16_p else 1)
                )
                dw = _pools.enter_context(tc.tile_pool(name="dw", bufs=1))
                return bm, dw

            if f16_x:
                bm, dw = _open_bmdw()

            # ---------------- stage A: q/k/v projections ----------------
            with (
                tc.tile_pool(name="aw", bufs=1) as aw,
                tc.tile_pool(name="ax", bufs=2) as ax,
                tc.tile_pool(name="aps", bufs=2, space="PSUM") as aps,
            ):
                wq_sb = aw.tile([P, KD, E], XDT)
                wk_sb = aw.tile([P, KD, E], XDT)
                wv_sb = aw.tile([P, KD, E], XDT)
                # SP issues DMAs in order: interleave weights with the first
                # x chunk so the q-projection's operands land first
                nc.sync.dma_start(out=wq_sb[:], in_=wqT_r[:])

                for cs in range(NCS):
                    ssl = slice(cs * CS, (cs + 1) * CS)
                    xq_sb = ax.tile([P, KD, CS], XDT, tag="xq")
                    xk_sb = ax.tile([P, KD, CS], XDT, tag="xk")
                    xv_sb = ax.tile([P, KD, CS], XDT, tag="xv")
                    nc.sync.dma_start(out=xq_sb[:], in_=xqT_r[:, :, ssl])
                    if cs == 0:
                        nc.sync.dma_start(
                            out=bqT_sb[:],
                            in_=bqT.rearrange("(t p) o -> p (t o)", p=P),
                        )
                        nc.sync.dma_start(
                            out=bkT_sb[:],
                            in_=bkT.rearrange("(t p) o -> p (t o)", p=P),
                        )
                        nc.sync.dma_start(out=bv_sb[:], in_=bv[:])
                        nc.sync.dma_start(out=onesx_sb[:], in_=ones_x[:])
                        nc.sync.dma_start(out=wk_sb[:], in_=wkT_r[:])
                    nc.sync.dma_start(out=xk_sb[:], in_=xkT_r[:, :, ssl])
                    if cs == 0:
                        nc.sync.dma_start(out=wv_sb[:], in_=wvT_r[:])
                    nc.sync.dma_start(out=xv_sb[:], in_=xvT_r[:, :, ssl])
                    if cs == 1:
                        nc.sync.dma_start(out=ones_sb[:], in_=ones_r[:])

                    for t in range(KE):
                        esl = slice(t * P, (t + 1) * P)
                        psq = aps.tile([P, CS], F32, tag="psq")
                        for k in range(KD):
                            nc.tensor.matmul(
                                psq[:], wq_sb[:, k, esl], xq_sb[:, k, :],
                                start=(k == 0), stop=(k == KD - 1),
                            )
                        nc.scalar.activation(
                            qT_sb[:, t, ssl],
                            psq[:],
                            mybir.ActivationFunctionType.Identity,
                            bias=bqT_sb[:, t : t + 1],
                        )

                        psk = aps.tile([P, CS], F32, tag="psk")
                        for k in range(KD):
                            nc.tensor.matmul(
                                psk[:], wk_sb[:, k, esl], xk_sb[:, k, :],
                                start=(k == 0), stop=(k == KD - 1),
                            )
                        nc.scalar.activation(
                            kT_sb[:, t, ssl],
                            psk[:],
                            mybir.ActivationFunctionType.Identity,
                            bias=bkT_sb[:, t : t + 1],
                        )

                    for st in range(CS // P):
                        stg = cs * (CS // P) + st
                        psv = aps.tile([P, E], F32, tag="psv")
                        for k in range(KD):
                            nc.tensor.matmul(
                                psv[:],
                                xv_sb[:, k, st * P : (st + 1) * P],
                                wv_sb[:, k, :],
                                start=(k == 0), stop=False,
                            )
                        nc.tensor.matmul(
                            psv[:], onesx_sb[:, :P], bv_sb[:],
                            start=False, stop=True,
                        )
                        nc.vector.tensor_copy(
                            v_sb[:, stg, :, 0:DK],
                            psv[:].rearrange("p (h d) -> p h d", h=H),
                        )

            # ---------------- stage B: attention ----------------
            if not f16_x:
                bm, dw = _open_bmdw()
            with (
                _pools,
                tc.tile_pool(name="be", bufs=4) as be,
                tc.tile_pool(name="bp", bufs=4) as bp,
                tc.tile_pool(name="bs", bufs=2) as bsc,
                tc.tile_pool(name="do", bufs=3) as do,
                tc.tile_pool(name="psS", bufs=2, space="PSUM") as psS,
                tc.tile_pool(name="psX", bufs=1, space="PSUM") as psX,
            ):
                wo_sb = dw.tile([P, KE, D], F32R)
                nc.sync.dma_start(out=wo_sb[:], in_=woT_r[:])
                # issue every mask load up front: the SP DMA queue is
                # in-order, so a load placed after cq=0's store traffic would
                # otherwise stall cq=1's whole exp/mult pipeline
                mask_tiles = []
                for cq in range(NCQ):
                    q0 = cq * CQ
                    mask_sb = bm.tile([P, SK, CQ], PDT, tag="mask")
                    nc.sync.dma_start(
                        out=mask_sb[:], in_=maskT_r[:, :, q0 : q0 + CQ]
                    )
                    mask_tiles.append(mask_sb)
                for cq in range(NCQ):
                    q0 = cq * CQ
                    mask_sb = mask_tiles[cq]
                    for hp in range(H // 2):
                        xa0 = psX.tile([DK1, CQ], F32, tag="xa0")
                        xa1 = psX.tile([DK1, CQ], F32, tag="xa1")
                        xa = [xa0, xa1]
                        for sk in range(SK):
                            # issue both heads' score matmuls back-to-back so
                            # the K=64 pair lands in distinct PE row groups
                            # (rows 0-63 / 64-127) and runs concurrently
                            ps_sA = psS.tile([P, CQ], F32, tag="s")
                            ps_sB = psS.tile([P, CQ], F32, tag="s")
                            ps_s = [ps_sA, ps_sB]
                            for h2 in range(2):
                                psl = slice(64 * h2, 64 * (h2 + 1))
                                for n in range(CQ // NB):
                                    nc.tensor.matmul(
                                        ps_s[h2][:, n * NB : (n + 1) * NB],
                                        kT_sb[psl, hp, sk * P : (sk + 1) * P],
                                        qT_sb[psl, hp, q0 + n * NB : q0 + (n + 1) * NB],
                                        start=True, stop=True,
                                    )
                            for h2 in range(2):
                                h = 2 * hp + h2
                                exp_sb = be.tile([P, CQ], PDT, tag="exp")
                                nc.scalar.activation(exp_sb[:], ps_s[h2][:], EXP)
                                pT = bp.tile([P, CQ], PDT, tag="pT")
                                nc.vector.tensor_mul(
                                    pT[:], exp_sb[:], mask_sb[:, sk, :]
                                )
                                for n in range(CQ // NB):
                                    nc.tensor.matmul(
                                        xa[h2][:, n * NB : (n + 1) * NB],
                                        v_sb[:, sk, h, :],
                                        pT[:, n * NB : (n + 1) * NB],
                                        start=(sk == 0), stop=(sk == SK - 1),
                                    )
                        for h2 in range(2):
                            # 1/denom via exp(-ln(d)) on ACT: Ln/Exp/Copy all
                            # live in one table set -> no table reloads, and
                            # it reads the PSUM denominator row directly.
                            rc1 = bsc.tile([P, CQ], F32, tag="rc1")
                            rc2 = bsc.tile([P, CQ], F32R, tag="rc2")
                            nc.scalar.activation(
                                rc1[64:65, :],
                                xa[h2][64:65, :],
                                mybir.ActivationFunctionType.Ln,
                            )
                            nc.scalar.activation(
                                rc2[64:65, :],
                                rc1[64:65, :],
                                EXP,
                                scale=-1.0,
                            )
                            bc_ps = psS.tile([64, CQ], F32, tag="s")
                            for n in range(CQ // NB):
                                nc.tensor.matmul(
                                    bc_ps[:, n * NB : (n + 1) * NB],
                                    ones_sb[64:65, :],
                                    rc2[64:65, n * NB : (n + 1) * NB],
                                    start=True, stop=True,
                                )
                            bc_sb = bsc.tile([64, CQ], F32, tag="bc")
                            nc.vector.tensor_copy(bc_sb[:], bc_ps[:])
                            nc.vector.tensor_mul(
                                xattT_sb[64 * h2 : 64 * (h2 + 1), hp, q0 : q0 + CQ],
                                xa[h2][0:DK, :],
                                bc_sb[:],
                            )

                    # ---- output projection for this sq-chunk (overlaps the
                    # next chunk's attention work) ----
                    for m in range(D // P):
                        msl = slice(m * P, (m + 1) * P)
                        o_sb = do.tile([P, CQ], F32, tag="osb")
                        for n in range(CQ // NB):
                            nsl_l = slice(n * NB, (n + 1) * NB)
                            nsl_g = slice(q0 + n * NB, q0 + (n + 1) * NB)
                            ps_o = psS.tile([P, NB], F32, tag="s")
                            for kk in range(KE):
                                nc.tensor.matmul(
                                    ps_o[:],
                                    wo_sb[:, kk, msl],
                                    xattT_sb[:, kk, nsl_g],
                                    start=(kk == 0), stop=(kk == KE - 1),
                                )
                            if (m + n) % 2 == 0:
                                nc.vector.tensor_copy(o_sb[:, nsl_l], ps_o[:])
                            else:
                                nc.scalar.copy(out=o_sb[:, nsl_l], in_=ps_o[:])
                        nc.gpsimd.dma_start(
                            out=outT_r[:, m, q0 : q0 + CQ], in_=o_sb[:]
                        )

    _split_multiwait(nc, 1)
    return nc


# ---------------------------------------------------------------- host side

B, S_FULL, D_FULL, H_FULL = 2, 2048, 1024, 16
DK_FULL = D_FULL // H_FULL
N_CORES = 8
GROUPS = N_CORES // B  # head-groups per batch
EG = D_FULL // GROUPS  # e-columns per core

_NC_CACHE = {}


def _get_program():
    key = "full"
    if key not in _NC_CACHE:
        _NC_CACHE[key] = build_program(D=D_FULL, S=S_FULL, E=EG, DK=DK_FULL)
    return _NC_CACHE[key]


def _cast(a, f16):
    a = np.ascontiguousarray(a, dtype=np.float32)
    if f16:
        import ml_dtypes

        return a.astype(ml_dtypes.bfloat16)
    return a


LAST_RES = None


def kernel(query, key, value, softmask, Wq, bq, Wk, bk, Wv, bv, Wo, bo, _trace=False):
    global LAST_RES
    from concourse.bass_utils import run_bass_kernel_spmd

    nc = _get_program()
    scale = 1.0 / np.sqrt(np.float32(DK_FULL))

    in_maps = []
    for c in range(N_CORES):
        b, g = c // GROUPS, c % GROUPS
        es = slice(g * EG, (g + 1) * EG)
        m = {
            "xqT": _cast(query[b].T, F16_X),
            "xkT": _cast(key[b].T, F16_X),
            "xvT": _cast(value[b].T, F16_X),
            "maskT": _cast(softmask[b].T + 1e-30, F16_P),
            "wqT": _cast(Wq[es, :].T * scale, F16_X),
            "wkT": _cast(Wk[es, :].T, F16_X),
            "wvT": _cast(Wv[es, :].T, F16_X),
            "woT": _cast(Wo[:, es].T, False),
            "bqT": _cast(bq[es, None] * scale, False),
            "bkT": _cast(bk[es, None], False),
            "bv": _cast(bv[None, es], F16_X),
            "ones_r": np.ones((128, 64), np.float32),
            "ones_x": _cast(np.ones((1, 512)), F16_X),
            "ones_p": _cast(np.ones((1, 128)), F16_P),
        }
        in_maps.append(m)

    res = run_bass_kernel_spmd(
        nc, in_maps, core_ids=list(range(N_CORES)), trace=_trace
    )
    LAST_RES = res

    out = np.zeros((B, S_FULL, D_FULL), dtype=np.float32)
    for c in range(N_CORES):
        b = c // GROUPS
        out[b] += res.results[c]["outT"].T
    out += np.asarray(bo, dtype=np.float32)[None, None, :]
    return out
